# revision 1
# baseline (speedup 1.0000x reference)
"""Trainium2 Bass kernel for the vq_codebook CCE loss.

Reference computation (live dataflow only):
    d2[c,b,p] = ||outputs[b] - clusters[c,p]||^2
    p*(b)     = argmin_p d2[tc_b, b, p]
    t         = mean_{b,f} (outputs[b,f] - clusters[tc_b, p*(b), f])^2
              = (1/(B*F)) * sum_b min_p d2[tc_b, b, p]
    out       = ALPHA*t + BETA*(1 - t)

Device strategy (8 NeuronCores, SPMD):
  - Classes padded 200 -> 208 and sharded 26 per core; outputs replicated.
  - Each core computes s[b,j] = c2[j] - 2*x[b]·c[j] for its 832 prototypes on
    the PE (fp8 operands, f32 PSUM; c2 enters as a rank-1 bf16 matmul with a
    ones lhsT), then a windowed min over each class's 32 prototypes (DVE),
    then selects the target class per row with a precomputed iota==target
    one-hot mask and a multiply+reduce.
  - ||x||^2 is computed on-device for the core's own 256-row slice.
  - Host combines: t = (sum x2 + sum selected_min)/(B*F).
  - Loop runs in 4 waves of 8 single-bank PSUM groups so the PE starts as
    soon as the first contraction chunk lands; DMAs are merged (few issues)
    and dependency-chained so chunk 0 completes at full bandwidth first.

fp8 notes: e4m3 quantization perturbs distances ~0.3%; the argmin can flip
between near-tied prototypes, which moves the mean-min-distance t by <0.5%.
The returned loss is ALPHA*t + BETA*(1-t) with ALPHA=BETA so the t-dependence
cancels to f32 rounding; rel err vs the f32 reference stays ~1e-7.
"""

import numpy as np
import ml_dtypes  # noqa: F401  (np dtype registry for bf16/fp8)
from contextlib import ExitStack

import concourse.tile as tile
from concourse import bacc, mybir
from concourse.tile import add_dep_helper
from concourse.bass_utils import run_bass_kernel_spmd

ALPHA = 5.0
BETA = 5.0

B, F, C, P = 2048, 768, 200, 32
NCORES = 8
CPAD = 208                # padded class count
CC = CPAD // NCORES       # 26 classes per core
JPC = CC * P              # 832 prototype columns per core
NJT, JT = 2, 416          # j tiles per core (13 classes each)
NFC = 6                   # contraction chunks over F=768
NBT = B // 128            # 16 batch tiles
OCT = 8                   # psum groups per wave
BSL = B // NCORES         # 256 rows per core for ||x||^2

F32 = mybir.dt.float32
BF16 = mybir.dt.bfloat16
KDT = mybir.dt.float8e4   # contraction operand dtype
AX = mybir.AxisListType
OP = mybir.AluOpType

_prog_cache = {}


def _build_program():
    if "nc" in _prog_cache:
        return _prog_cache["nc"]

    nc = bacc.Bacc(
        "TRN2", target_bir_lowering=False, debug=False, num_devices=NCORES,
        enable_asserts=False, enable_partition_id=False,
    )

    a_t = nc.dram_tensor("a_t", [128, NFC, B], KDT, kind="ExternalInput").ap()
    cg = nc.dram_tensor("cg", [128, NFC, JPC], KDT, kind="ExternalInput").ap()
    # [1, :JPC] = c2 row (bf16), then [1, 128] of ones
    miscb = nc.dram_tensor("miscb", [1, JPC + 128], BF16, kind="ExternalInput").ap()
    # [:, :NBT] = target class per row tile, [:, NBT:] = global class ids
    miscf = nc.dram_tensor("miscf", [128, NBT + CC], F32, kind="ExternalInput").ap()
    outn = nc.dram_tensor("outn", [128, 2 * F], BF16, kind="ExternalInput").ap()
    out = nc.dram_tensor("out", [128, NBT + 2], F32, kind="ExternalOutput").ap()

    with tile.TileContext(nc) as tc, ExitStack() as ctx:
        const = ctx.enter_context(tc.tile_pool(name="const", bufs=1))
        psum = ctx.enter_context(tc.tile_pool(name="psum", bufs=8, space="PSUM"))
        work = ctx.enter_context(tc.tile_pool(name="work", bufs=4))

        a_sb = const.tile([128, NFC * B], KDT, name="a_sb", tag="a")
        cg_sb = const.tile([128, NFC * JPC], KDT, name="cg_sb", tag="cgs")
        mb_sb = const.tile([1, JPC + 128], BF16, name="mb_sb", tag="mb")
        mf_sb = const.tile([128, NBT + CC], F32, name="mf_sb", tag="mf")
        outn_sb = const.tile([128, 2 * F], BF16, name="outn_sb", tag="outn")
        mask_sb = const.tile([128, NBT * CC], F32, name="mask_sb", tag="mask")
        m_sb = const.tile([128, NBT * CC], F32, name="m_sb", tag="m")
        res = const.tile([128, NBT + 2], F32, name="res", tag="res")

        c2_row = mb_sb[:, 0:JPC]
        ones = mb_sb[:, JPC : JPC + 128]

        # --- DMAs: stream exactly what wave 0 needs first ---
        HB = B // 2  # first 8 b-tiles of each chunk
        a_v = a_sb[:].rearrange("p (c b) -> p c b", c=NFC)
        cg_v = cg_sb[:].rearrange("p (c j) -> p c j", c=NFC)
        d_a0a = nc.sync.dma_start(a_v[:, 0, 0:HB], a_t[:, 0, 0:HB])
        d_cg0a = nc.sync.dma_start(cg_v[:, 0, 0:JT], cg[:, 0, 0:JT])
        d_mb = nc.sync.dma_start(mb_sb[:], miscb)
        d_mf = nc.sync.dma_start(mf_sb[:], miscf)
        d_af1 = nc.sync.dma_start(a_v[:, 1:2, 0:HB], a_t[:, 1:2, 0:HB])
        d_cgf = nc.sync.dma_start(cg_v[:, 1:NFC, 0:JT], cg[:, 1:NFC, 0:JT])
        d_af2 = nc.sync.dma_start(a_v[:, 2:NFC, 0:HB], a_t[:, 2:NFC, 0:HB])
        d_cgs = nc.sync.dma_start(cg_v[:, :, JT:JPC], cg[:, :, JT:JPC])
        d_as = nc.sync.dma_start(a_v[:, :, HB:B], a_t[:, :, HB:B])
        add_dep_helper(d_af1.ins, d_a0a.ins, reason="chunk0 first")
        add_dep_helper(d_cgf.ins, d_cg0a.ins, reason="chunk0 first")
        add_dep_helper(d_af2.ins, d_af1.ins, reason="chunk order")
        add_dep_helper(d_cgs.ins, d_af2.ins, reason="jt1 after wave0 set")
        add_dep_helper(d_as.ins, d_af2.ins, reason="oct1 after wave0 set")
        d_on = nc.sync.dma_start(outn_sb[:], outn)
        add_dep_helper(d_on.ins, d_as.ins, reason="outn only needed at tail")

        # --- one-hot masks precomputed in the DMA shadow ---
        for bh in range(NBT):
            nc.gpsimd.tensor_scalar(
                out=mask_sb[:, bh * CC : (bh + 1) * CC],
                in0=mf_sb[:, NBT : NBT + CC],
                scalar1=mf_sb[:, bh : bh + 1], scalar2=None,
                op0=OP.is_equal,
            )

        # --- waves of single-bank psum groups (last split for a short tail) ---
        WAVES = [
            (0, range(0, 8)),
            (1, range(0, 8)),
            (0, range(8, 16)),
            (1, range(8, 12)),
            (1, range(12, 14)),
            (1, range(14, 16)),
        ]
        for wave, (jt, bhs) in enumerate(WAVES):
            if wave == 3:
                # ||x||^2 for this core's 256-row slice, in the shadow of
                # the last wave's matmuls.
                for t in range(2):
                    sq = work.tile([128, F], F32, name="sq", tag="sq")
                    xs = outn_sb[:, t * F : (t + 1) * F]
                    nc.vector.tensor_tensor(
                        out=sq[:], in0=xs, in1=xs, op=OP.mult
                    )
                    nc.vector.tensor_reduce(
                        out=res[:, NBT + t : NBT + t + 1], in_=sq[:],
                        axis=AX.X, op=OP.add,
                    )
            bhs = list(bhs)
            pss = [
                psum.tile([128, 512], F32, name="ps", tag="ps")
                for _ in bhs
            ]
            for c in range(NFC):
                for i, bh in enumerate(bhs):
                    nc.tensor.matmul(
                        pss[i][:, 0:JT],
                        lhsT=a_sb[:, c * B + bh * 128 : c * B + (bh + 1) * 128],
                        rhs=cg_sb[:, c * JPC + jt * JT : c * JPC + (jt + 1) * JT],
                        start=(c == 0),
                        stop=False,
                    )
            for i, bh in enumerate(bhs):
                nc.tensor.matmul(
                    pss[i][:, 0:JT],
                    lhsT=ones,
                    rhs=c2_row[:, jt * JT : (jt + 1) * JT],
                    start=False, stop=True,
                )
            for i, bh in enumerate(bhs):
                nc.vector.tensor_reduce(
                    out=m_sb[:, bh * CC + jt * 13 : bh * CC + jt * 13 + 13],
                    in_=pss[i][:, 0:JT].rearrange("p (w k) -> p w k", k=P),
                    axis=AX.X,
                    op=OP.min,
                )
            if jt == 1:
                for bh in bhs:
                    junk = work.tile([128, CC], F32, name="junk", tag="junk")
                    nc.gpsimd.tensor_tensor(
                        out=junk[:],
                        in0=mask_sb[:, bh * CC : (bh + 1) * CC],
                        in1=m_sb[:, bh * CC : (bh + 1) * CC], op=OP.mult,
                    )
                    nc.vector.tensor_reduce(
                        out=res[:, bh : bh + 1], in_=junk[:],
                        axis=AX.X, op=OP.add,
                    )

        nc.sync.dma_start(out, res[:])

    nc.compile()
    _prog_cache["nc"] = nc
    return nc


def _prep_inputs(outputs, clusters, target_classes):
    outputs = np.ascontiguousarray(np.asarray(outputs, dtype=np.float32))
    clusters = np.ascontiguousarray(np.asarray(clusters, dtype=np.float32))
    tc_np = np.asarray(target_classes)

    np_k = mybir.dt.np(KDT)
    np_b = mybir.dt.np(BF16)

    flat = clusters.reshape(C * P, F)
    cgt = np.zeros((F, CPAD * P), np.float32)
    cgt[:, : C * P] = flat.T
    c2 = np.zeros(CPAD * P, np.float32)
    c2[: C * P] = (flat * flat).sum(axis=1)

    # lhsT chunks: a_t[p, c, b] = -2 * outputs[b, c*128+p]
    a_t = np.ascontiguousarray(
        (-2.0 * outputs.T).astype(np_k).reshape(NFC, 128, B).transpose(1, 0, 2)
    )
    tct = tc_np.astype(np.float32).reshape(NBT, 128).T

    in_maps = []
    for i in range(NCORES):
        sl = cgt[:, i * JPC : (i + 1) * JPC]
        cg_i = np.ascontiguousarray(
            sl.astype(np_k).reshape(NFC, 128, JPC).transpose(1, 0, 2)
        )
        miscb_i = np.zeros((1, JPC + 128), np_b)
        miscb_i[0, :JPC] = c2[i * JPC : (i + 1) * JPC].astype(np_b)
        miscb_i[0, JPC:] = np.ones(128, np_b)
        miscf_i = np.empty((128, NBT + CC), np.float32)
        miscf_i[:, :NBT] = tct
        miscf_i[:, NBT:] = np.arange(i * CC, (i + 1) * CC, dtype=np.float32)
        outn_i = np.ascontiguousarray(
            outputs[i * BSL : (i + 1) * BSL].astype(np_b).reshape(2, 128, F)
            .transpose(1, 0, 2).reshape(128, 2 * F)
        )
        in_maps.append(
            {
                "a_t": a_t,
                "cg": cg_i,
                "miscb": miscb_i,
                "miscf": np.ascontiguousarray(miscf_i),
                "outn": outn_i,
            }
        )
    return in_maps


def _finish(results):
    s = 0.0
    for r in results:
        s += float(r["out"].astype(np.float64).sum())
    t = np.float32(s / (B * F))
    ans = np.float32(ALPHA) * t + np.float32(BETA) * (np.float32(1.0) - t)
    return np.asarray(ans, dtype=np.float32)


def kernel(outputs, clusters, target_classes, _run_kwargs=None):
    nc = _build_program()
    in_maps = _prep_inputs(outputs, clusters, target_classes)
    kw = _run_kwargs or {}
    res = run_bass_kernel_spmd(nc, in_maps, list(range(NCORES)), **kw)
    ans = _finish(res.results)
    if _run_kwargs is not None:
        kernel.last_result = res
    return ans


if __name__ == "__main__":
    rng = np.random.default_rng(0)
    o = rng.standard_normal((B, F), dtype=np.float32)
    cl = rng.standard_normal((C, P, F), dtype=np.float32)
    t = rng.integers(0, C, size=(B,)).astype(np.int32)
    print(kernel(o, cl, t))



# revision 15
# speedup vs baseline: 2.0704x; 2.0704x over previous
"""Trainium2 Bass kernel for the vq_codebook CCE loss.

Live dataflow of the reference:
    t   = (1/(B*F)) * sum_b min_p ||outputs[b] - clusters[tc_b, p]||^2
    out = ALPHA*t + BETA*(1 - t)
Only the TARGET class's prototype distances feed the loss (the wrong-class
branch of the reference is dead code), so per batch row only 32 of the
6400 prototype distances are live.

Strategy (8 NeuronCores, SPMD):
  - Host sorts rows by target class (stable) and splits the sorted batch
    into 16 tiles of 128 rows.  Each tile's rows span a small contiguous
    class range (<=16 classes for random data), so a single 512-column
    PSUM bank holds every prototype column any of its rows needs.
  - Each core takes 2 tiles.  Per tile: 3 fp8 DoubleRow matmuls (256
    contraction rows each) compute -2*x.c for the gathered columns, plus
    one DoubleRow rank-2 matmul adds ||c||^2 (split 16*h + r, both fp8,
    abs err <= 2).  DVE takes a windowed min over each class's 32
    prototypes, then a fused mask-select+sum picks each row's own class.
  - ||x||^2 comes from a single Scalar-engine Square pass with accum_out
    over the core's fp8 x slice.
  - Host combines: t = (sum x2 + sum selected_min)/(B*F).

fp8 e4m3 quantization moves t by ~0.03% (validated off-device vs f64).
"""

import numpy as np
import ml_dtypes  # noqa: F401  (np dtype registry for bf16/fp8)
from contextlib import ExitStack

import concourse.tile as tile
from concourse import bacc, mybir
from concourse.tile import add_dep_helper
from concourse.bass_utils import run_bass_kernel_spmd

ALPHA = 5.0
BETA = 5.0

B, F, C, P = 2048, 768, 200, 32
NCORES = 8
NT = B // 128            # 16 row tiles of 128 sorted rows
TPC = NT // NCORES       # 2 tiles per core
K3 = F // 256            # 3 DoubleRow contraction chunks
RPC = 128 * TPC          # 256 rows per core

F32 = mybir.dt.float32
BF16 = mybir.dt.bfloat16
KDT = mybir.dt.float8e4
AX = mybir.AxisListType
OP = mybir.AluOpType

_prog_cache = {}

import os
V_C2 = os.environ.get("KV_C2", "dr")      # dr | bf16
V_X2 = os.environ.get("KV_X2", "act")     # act | off
V_MM = os.environ.get("KV_MM", "dr")      # dr | plain
# NOTE: tensor_tensor_reduce crashes the exec unit on this HW (bisected);
# keep the split gpsimd-mult + vector-reduce form.
V_SEL = os.environ.get("KV_SEL", "split")   # ttr | split


def _build_program(nb):
    """nb = PSUM banks per tile (1 unless some tile spans >16 classes)."""
    key = ("nc", nb, V_C2, V_X2, V_MM, V_SEL)
    if key in _prog_cache:
        return _prog_cache[key]

    ncol = 512 * nb          # prototype columns per tile
    nw = ncol // 32          # class windows per tile

    nc = bacc.Bacc(
        "TRN2", target_bir_lowering=False, debug=False, num_devices=NCORES,
        enable_asserts=False, enable_partition_id=False,
    )

    xa = nc.dram_tensor("xa", [128, K3 * 2 * RPC], KDT, kind="ExternalInput").ap()
    cg = nc.dram_tensor("cg", [128, K3 * TPC * 2 * ncol], KDT, kind="ExternalInput").ap()
    # per tile: h row (ncol), r row (ncol); then lhsT consts (16.0)*128, (1.0)*128
    mb = nc.dram_tensor("mb", [1, TPC * 2 * ncol + 256], KDT, kind="ExternalInput").ap()
    m2 = nc.dram_tensor("m2", [1, TPC * ncol + 128], BF16, kind="ExternalInput").ap()
    mk = nc.dram_tensor("mk", [128, TPC * nw], F32, kind="ExternalInput").ap()
    out = nc.dram_tensor("out", [128, 3], F32, kind="ExternalOutput").ap()

    DR = mybir.MatmulPerfMode.DoubleRow

    with tile.TileContext(nc) as tc, ExitStack() as ctx:
        const = ctx.enter_context(tc.tile_pool(name="const", bufs=1))
        psum = ctx.enter_context(tc.tile_pool(name="psum", bufs=2 * nb, space="PSUM"))

        xa_sb = const.tile([128, K3 * 2 * RPC], KDT, name="xa_sb", tag="xa")
        cg_sb = const.tile([128, K3 * TPC * 2 * ncol], KDT, name="cg_sb", tag="cg")
        mb_sb = const.tile([1, TPC * 2 * ncol + 256], KDT, name="mb_sb", tag="mb")
        m2_sb = const.tile([1, TPC * ncol + 128], BF16, name="m2_sb", tag="m2")
        mk_sb = const.tile([128, TPC * nw], F32, name="mk_sb", tag="mk")
        mwin = const.tile([128, TPC * nw], F32, name="mwin", tag="mw")
        junk = const.tile([128, TPC * nw], F32, name="junk", tag="jk")
        sq = const.tile([128, K3 * 2 * RPC], BF16, name="sq", tag="sq")
        res = const.tile([128, 3], F32, name="res", tag="res")

        xa_v = xa_sb[:].rearrange("p (k s r) -> p k s r", k=K3, s=2)
        cg_v = cg_sb[:].rearrange("p (k t s j) -> p k t s j", k=K3, t=TPC, s=2)
        mb_v = mb_sb[:, 0 : TPC * 2 * ncol].rearrange(
            "p (t s j) -> p t s j", t=TPC, s=2
        )
        ones2 = mb_sb[:, TPC * 2 * ncol :].rearrange("p (s r) -> p s r", s=2)

        # --- DMAs: chunk 0 first so the PE can start early ---
        d_mb = nc.sync.dma_start(mb_sb[:], mb)
        d_m2 = nc.sync.dma_start(m2_sb[:], m2)
        d_mk = nc.sync.dma_start(mk_sb[:], mk)
        xa_f = xa_sb[:].rearrange("p (k x) -> p k x", k=K3)
        xa_d = xa.rearrange("p (k x) -> p k x", k=K3)
        cg_f = cg_sb[:].rearrange("p (k x) -> p k x", k=K3)
        cg_d = cg.rearrange("p (k x) -> p k x", k=K3)
        d_xa0 = nc.sync.dma_start(xa_f[:, 0, :], xa_d[:, 0, :])
        d_xar = nc.sync.dma_start(xa_f[:, 1:, :], xa_d[:, 1:, :])
        add_dep_helper(d_xar.ins, d_xa0.ins, reason="chunk0 first")
        d_cg = [nc.sync.dma_start(cg_f[:, k, :], cg_d[:, k, :]) for k in range(K3)]
        add_dep_helper(d_cg[1].ins, d_cg[0].ins, reason="chunk order")
        add_dep_helper(d_cg[2].ins, d_cg[1].ins, reason="chunk order")

        # --- Sum x^2 on the Scalar engine in the DMA/PE shadow ---
        if V_X2 == "act":
            nc.scalar.activation(
                out=sq[:], in_=xa_sb[:],
                func=mybir.ActivationFunctionType.Square,
                accum_out=res[:, 2:3],
            )
        else:
            nc.gpsimd.memset(res[:, 2:3], 0.0)

        # --- PE: per tile, 3 DoubleRow chunks + rank-2 c2 add ---
        pss = [psum.tile([128, ncol], F32, name="ps", tag="ps") for _ in range(TPC)]
        for k in range(K3):
            for t in range(TPC):
                for b in range(nb):
                    if V_MM == "dr":
                        nc.tensor.matmul(
                            pss[t][:, b * 512 : (b + 1) * 512],
                            lhsT=xa_v[:, k, :, t * 128 : (t + 1) * 128],
                            rhs=cg_v[:, k, t, :, b * 512 : (b + 1) * 512],
                            perf_mode=DR,
                            start=(k == 0),
                            stop=False,
                        )
                    else:
                        for s in range(2):
                            nc.tensor.matmul(
                                pss[t][:, b * 512 : (b + 1) * 512],
                                lhsT=xa_v[:, k, s, t * 128 : (t + 1) * 128],
                                rhs=cg_v[:, k, t, s, b * 512 : (b + 1) * 512],
                                start=(k == 0 and s == 0),
                                stop=False,
                            )
        for t in range(TPC):
            for b in range(nb):
                if V_C2 == "dr":
                    nc.tensor.matmul(
                        pss[t][:, b * 512 : (b + 1) * 512],
                        lhsT=ones2,
                        rhs=mb_v[:, t, :, b * 512 : (b + 1) * 512],
                        perf_mode=DR,
                        start=False,
                        stop=True,
                    )
                else:
                    nc.tensor.matmul(
                        pss[t][:, b * 512 : (b + 1) * 512],
                        lhsT=m2_sb[:, TPC * ncol : TPC * ncol + 128],
                        rhs=m2_sb[:, t * ncol + b * 512 : t * ncol + (b + 1) * 512],
                        start=False,
                        stop=True,
                    )

        # --- DVE: windowed min over 32 prototypes, then mask-select+sum ---
        for t in range(TPC):
            nc.vector.tensor_reduce(
                out=mwin[:, t * nw : (t + 1) * nw],
                in_=pss[t][:].rearrange("p (w x) -> p w x", x=P),
                axis=AX.X,
                op=OP.min,
            )
            if V_SEL == "ttr":
                nc.vector.tensor_tensor_reduce(
                    out=junk[:, t * nw : (t + 1) * nw],
                    in0=mwin[:, t * nw : (t + 1) * nw],
                    in1=mk_sb[:, t * nw : (t + 1) * nw],
                    scale=1.0,
                    scalar=0.0,
                    op0=OP.mult,
                    op1=OP.add,
                    accum_out=res[:, t : t + 1],
                )
            else:
                nc.gpsimd.tensor_tensor(
                    out=junk[:, t * nw : (t + 1) * nw],
                    in0=mwin[:, t * nw : (t + 1) * nw],
                    in1=mk_sb[:, t * nw : (t + 1) * nw],
                    op=OP.mult,
                )
                nc.vector.tensor_reduce(
                    out=res[:, t : t + 1],
                    in_=junk[:, t * nw : (t + 1) * nw],
                    axis=AX.X,
                    op=OP.add,
                )

        nc.sync.dma_start(out, res[:])

    nc.compile()
    _prog_cache[key] = nc
    return nc


def _prep_inputs(outputs, clusters, target_classes):
    outputs = np.ascontiguousarray(np.asarray(outputs, dtype=np.float32))
    clusters = np.ascontiguousarray(np.asarray(clusters, dtype=np.float32))
    tc_np = np.asarray(target_classes).astype(np.int64)

    np_k = mybir.dt.np(KDT)
    np_b = mybir.dt.np(BF16)

    order = np.argsort(tc_np, kind="stable")
    xs = outputs[order]
    tcs = tc_np[order]

    los = np.empty(NT, np.int64)
    spans = np.empty(NT, np.int64)
    for t in range(NT):
        seg = tcs[t * 128 : (t + 1) * 128]
        los[t] = seg.min()
        spans[t] = seg.max() - seg.min() + 1
    nb = max(1, int(-(-int(spans.max()) // 16)))
    ncol = 512 * nb
    nw = ncol // 32

    flat = clusters.reshape(C * P, F)
    c2 = (flat.astype(np.float64) ** 2).sum(axis=1).astype(np.float32)

    # -2x in fp8, laid out (p, k, s, r): feature = k*256 + s*128 + p
    a8 = np.clip(-2.0 * xs, -240, 240).astype(np_k)  # [B, F]

    in_maps = []
    for ci in range(NCORES):
        rows = slice(ci * RPC, (ci + 1) * RPC)
        xa_i = np.ascontiguousarray(
            a8[rows].T.reshape(K3, 2, 128, RPC).transpose(2, 0, 1, 3)
            .reshape(128, K3 * 2 * RPC)
        )

        cg_i = np.zeros((128, K3, TPC, 2, ncol), np_k)
        mb_i = np.zeros((1, TPC * 2 * ncol + 256), np_k)
        m2_i = np.zeros((1, TPC * ncol + 128), np_b)
        mk_i = np.zeros((128, TPC * nw), np.float32)
        for tt in range(TPC):
            t = ci * TPC + tt
            lo = int(los[t])
            hi = min(lo + nw, C)
            npro = (hi - lo) * P
            G = flat[lo * P : hi * P]                       # [npro, F]
            g8 = np.clip(G, -240, 240).astype(np_k)
            # (F, npro) -> (k, s, p, npro) -> (p, k, s, npro)
            cg_i[:, :, tt, :, :npro] = (
                g8.T.reshape(K3, 2, 128, npro).transpose(2, 0, 1, 3)
            )
            c2t = np.zeros(ncol, np.float32)
            c2t[:npro] = c2[lo * P : hi * P]
            h8 = np.clip(c2t / 16.0, -240, 240).astype(np_k)
            r8 = np.clip(c2t - 16.0 * h8.astype(np.float32), -240, 240).astype(np_k)
            mb_i[0, tt * 2 * ncol : tt * 2 * ncol + ncol] = h8
            mb_i[0, tt * 2 * ncol + ncol : (tt + 1) * 2 * ncol] = r8
            m2_i[0, tt * ncol : (tt + 1) * ncol] = c2t.astype(np_b)
            w = tcs[t * 128 : (t + 1) * 128] - lo           # [128] window idx
            mk_i[np.arange(128), tt * nw + w] = 1.0
        mb_i[0, TPC * 2 * ncol : TPC * 2 * ncol + 128] = np.float32(16.0).astype(np_k)
        mb_i[0, TPC * 2 * ncol + 128 :] = np.float32(1.0).astype(np_k)
        m2_i[0, TPC * ncol :] = np.float32(1.0).astype(np_b)

        in_maps.append(
            {
                "xa": xa_i,
                "cg": np.ascontiguousarray(cg_i.reshape(128, -1)),
                "mb": mb_i,
                "m2": m2_i,
                "mk": mk_i,
            }
        )
    return nb, in_maps


def _finish(results):
    s = 0.0
    for r in results:
        o = r["out"].astype(np.float64)
        s += o[:, 0].sum() + o[:, 1].sum() + o[:, 2].sum() / 4.0
    t = np.float32(s / (B * F))
    ans = np.float32(ALPHA) * t + np.float32(BETA) * (np.float32(1.0) - t)
    return np.asarray(ans, dtype=np.float32)


def kernel(outputs, clusters, target_classes, _run_kwargs=None):
    nb, in_maps = _prep_inputs(outputs, clusters, target_classes)
    nc = _build_program(nb)
    kw = _run_kwargs or {}
    res = run_bass_kernel_spmd(nc, in_maps, list(range(NCORES)), **kw)
    ans = _finish(res.results)
    if _run_kwargs is not None:
        kernel.last_result = res
    return ans


if __name__ == "__main__":
    rng = np.random.default_rng(0)
    o = rng.standard_normal((B, F), dtype=np.float32)
    cl = rng.standard_normal((C, P, F), dtype=np.float32)
    t = rng.integers(0, C, size=(B,)).astype(np.int32)
    print(kernel(o, cl, t))


# revision 19
# speedup vs baseline: 2.8968x; 1.3991x over previous
"""Trainium2 Bass kernel for the vq_codebook CCE loss.

Live dataflow of the reference:
    t   = (1/(B*F)) * sum_b min_p ||outputs[b] - clusters[tc_b, p]||^2
    out = ALPHA*t + BETA*(1 - t)
Only the TARGET class's prototype distances feed the loss (the wrong-class
branch of the reference is dead code), so per batch row only 32 of the
6400 prototype distances are live.

Strategy (8 NeuronCores, SPMD):
  - Host sorts rows by target class (stable) and splits the sorted batch
    into 16 tiles of 128 rows.  Each tile's rows span a small contiguous
    class range (<=16 classes for random data), so a single 512-column
    PSUM bank holds every prototype column any of its rows needs.
  - Each core takes 2 tiles.  Per tile: 3 fp8 DoubleRow matmuls (256
    contraction rows each) compute -2*x.c for the gathered columns, plus
    one DoubleRow rank-2 matmul adds ||c||^2 (split 16*h + r, both fp8,
    abs err <= 2).  DVE takes a windowed min over each class's 32
    prototypes, then a fused mask-select+sum picks each row's own class.
  - ||x||^2 comes from a single Scalar-engine Square pass with accum_out
    over the core's fp8 x slice.
  - Host combines: t = (sum x2 + sum selected_min)/(B*F).

fp8 e4m3 quantization moves t by ~0.03% (validated off-device vs f64).
"""

import numpy as np
import ml_dtypes  # noqa: F401  (np dtype registry for bf16/fp8)
from contextlib import ExitStack

import concourse.tile as tile
from concourse import bacc, mybir
from concourse.tile import add_dep_helper
from concourse.bass_utils import run_bass_kernel_spmd

ALPHA = 5.0
BETA = 5.0

B, F, C, P = 2048, 768, 200, 32
NCORES = 8
NT = B // 128            # 16 row tiles of 128 sorted rows
TPC = NT // NCORES       # 2 tiles per core
K3 = F // 256            # 3 DoubleRow contraction chunks
RPC = 128 * TPC          # 256 rows per core

F32 = mybir.dt.float32
BF16 = mybir.dt.bfloat16
KDT = mybir.dt.float8e4
AX = mybir.AxisListType
OP = mybir.AluOpType

_prog_cache = {}

import os
V_C2 = os.environ.get("KV_C2", "dr")      # dr | bf16
V_X2 = os.environ.get("KV_X2", "act")     # act | off
V_MM = os.environ.get("KV_MM", "dr")      # dr | plain
# NOTE: tensor_tensor_reduce crashes the exec unit on this HW (bisected);
# keep the split gpsimd-mult + vector-reduce form.
V_SEL = os.environ.get("KV_SEL", "split")   # ttr | split


def _build_program(nb):
    """nb = PSUM banks per tile (1 unless some tile spans >16 classes)."""
    key = ("nc", nb, V_C2, V_X2, V_MM, V_SEL)
    if key in _prog_cache:
        return _prog_cache[key]

    ncol = 512 * nb          # prototype columns per tile
    nw = ncol // 32          # class windows per tile

    nc = bacc.Bacc(
        "TRN2", target_bir_lowering=False, debug=False, num_devices=NCORES,
        enable_asserts=False, enable_partition_id=False,
    )

    xa = nc.dram_tensor("xa", [128, K3 * 2 * RPC], KDT, kind="ExternalInput").ap()
    cg = nc.dram_tensor("cg", [128, K3 * TPC * 2 * ncol], KDT, kind="ExternalInput").ap()
    # per tile: h row (ncol), r row (ncol); then lhsT consts (16.0)*128, (1.0)*128
    mb = nc.dram_tensor("mb", [1, TPC * 2 * ncol + 256], KDT, kind="ExternalInput").ap()
    m2 = (
        nc.dram_tensor("m2", [1, TPC * ncol + 128], BF16, kind="ExternalInput").ap()
        if V_C2 == "bf16" else None
    )
    mk = nc.dram_tensor("mk", [128, TPC * nw], F32, kind="ExternalInput").ap()
    out = nc.dram_tensor("out", [128, 3], F32, kind="ExternalOutput").ap()

    DR = mybir.MatmulPerfMode.DoubleRow

    with tile.TileContext(nc) as tc, ExitStack() as ctx:
        const = ctx.enter_context(tc.tile_pool(name="const", bufs=1))
        psum = ctx.enter_context(tc.tile_pool(name="psum", bufs=2 * nb, space="PSUM"))

        xa_sb = const.tile([128, K3 * 2 * RPC], KDT, name="xa_sb", tag="xa")
        cg_sb = const.tile([128, K3 * TPC * 2 * ncol], KDT, name="cg_sb", tag="cg")
        mb_sb = const.tile([1, TPC * 2 * ncol + 256], KDT, name="mb_sb", tag="mb")
        m2_sb = (
            const.tile([1, TPC * ncol + 128], BF16, name="m2_sb", tag="m2")
            if V_C2 == "bf16" else None
        )
        mk_sb = const.tile([128, TPC * nw], F32, name="mk_sb", tag="mk")
        mwin = const.tile([128, TPC * nw], F32, name="mwin", tag="mw")
        junk = const.tile([128, TPC * nw], F32, name="junk", tag="jk")
        sq = const.tile([128, K3 * 2 * RPC], BF16, name="sq", tag="sq")
        res = const.tile([128, 3], F32, name="res", tag="res")

        xa_v = xa_sb[:].rearrange("p (k s r) -> p k s r", k=K3, s=2)
        cg_v = cg_sb[:].rearrange("p (k t s j) -> p k t s j", k=K3, t=TPC, s=2)
        mb_v = mb_sb[:, 0 : TPC * 2 * ncol].rearrange(
            "p (t s j) -> p t s j", t=TPC, s=2
        )
        ones2 = mb_sb[:, TPC * 2 * ncol :].rearrange("p (s r) -> p s r", s=2)

        # --- DMAs: no dep chains (each chained link pays ~2us completion
        # latency).  cg streams on the sync HWDGE ring in chunk order; xa
        # and the small tensors ride the scalar HWDGE ring in parallel. ---
        cg_f = cg_sb[:].rearrange("p (k x) -> p k x", k=K3)
        cg_d = cg.rearrange("p (k x) -> p k x", k=K3)
        d_cg = [nc.sync.dma_start(cg_f[:, k, :], cg_d[:, k, :]) for k in range(K3)]
        nc.scalar.dma_start(xa_sb[:], xa)
        nc.scalar.dma_start(mb_sb[:], mb)
        if m2_sb is not None:
            nc.scalar.dma_start(m2_sb[:], m2)
        nc.scalar.dma_start(mk_sb[:], mk)

        # --- Sum x^2 on the Scalar engine in the DMA/PE shadow ---
        if V_X2 == "act":
            nc.scalar.activation(
                out=sq[:], in_=xa_sb[:],
                func=mybir.ActivationFunctionType.Square,
                accum_out=res[:, 2:3],
            )
        else:
            nc.gpsimd.memset(res[:, 2:3], 0.0)

        # --- PE: per tile, 3 DoubleRow chunks + rank-2 c2 add ---
        pss = [psum.tile([128, ncol], F32, name="ps", tag="ps") for _ in range(TPC)]
        for k in range(K3):
            for t in range(TPC):
                for b in range(nb):
                    if V_MM == "dr":
                        nc.tensor.matmul(
                            pss[t][:, b * 512 : (b + 1) * 512],
                            lhsT=xa_v[:, k, :, t * 128 : (t + 1) * 128],
                            rhs=cg_v[:, k, t, :, b * 512 : (b + 1) * 512],
                            perf_mode=DR,
                            start=(k == 0),
                            stop=False,
                        )
                    else:
                        for s in range(2):
                            nc.tensor.matmul(
                                pss[t][:, b * 512 : (b + 1) * 512],
                                lhsT=xa_v[:, k, s, t * 128 : (t + 1) * 128],
                                rhs=cg_v[:, k, t, s, b * 512 : (b + 1) * 512],
                                start=(k == 0 and s == 0),
                                stop=False,
                            )
        for t in range(TPC):
            for b in range(nb):
                if V_C2 == "dr":
                    nc.tensor.matmul(
                        pss[t][:, b * 512 : (b + 1) * 512],
                        lhsT=ones2,
                        rhs=mb_v[:, t, :, b * 512 : (b + 1) * 512],
                        perf_mode=DR,
                        start=False,
                        stop=True,
                    )
                else:
                    nc.tensor.matmul(
                        pss[t][:, b * 512 : (b + 1) * 512],
                        lhsT=m2_sb[:, TPC * ncol : TPC * ncol + 128],
                        rhs=m2_sb[:, t * ncol + b * 512 : t * ncol + (b + 1) * 512],
                        start=False,
                        stop=True,
                    )

        # --- DVE: windowed min over 32 prototypes, then mask-select+sum ---
        for t in range(TPC):
            nc.vector.tensor_reduce(
                out=mwin[:, t * nw : (t + 1) * nw],
                in_=pss[t][:].rearrange("p (w x) -> p w x", x=P),
                axis=AX.X,
                op=OP.min,
            )
            if V_SEL == "ttr":
                nc.vector.tensor_tensor_reduce(
                    out=junk[:, t * nw : (t + 1) * nw],
                    in0=mwin[:, t * nw : (t + 1) * nw],
                    in1=mk_sb[:, t * nw : (t + 1) * nw],
                    scale=1.0,
                    scalar=0.0,
                    op0=OP.mult,
                    op1=OP.add,
                    accum_out=res[:, t : t + 1],
                )
            else:
                nc.gpsimd.tensor_tensor(
                    out=junk[:, t * nw : (t + 1) * nw],
                    in0=mwin[:, t * nw : (t + 1) * nw],
                    in1=mk_sb[:, t * nw : (t + 1) * nw],
                    op=OP.mult,
                )
                nc.vector.tensor_reduce(
                    out=res[:, t : t + 1],
                    in_=junk[:, t * nw : (t + 1) * nw],
                    axis=AX.X,
                    op=OP.add,
                )

        nc.sync.dma_start(out, res[:])

    nc.compile()
    _prog_cache[key] = nc
    return nc


def _prep_inputs(outputs, clusters, target_classes):
    outputs = np.ascontiguousarray(np.asarray(outputs, dtype=np.float32))
    clusters = np.ascontiguousarray(np.asarray(clusters, dtype=np.float32))
    tc_np = np.asarray(target_classes).astype(np.int64)

    np_k = mybir.dt.np(KDT)
    np_b = mybir.dt.np(BF16)

    order = np.argsort(tc_np, kind="stable")
    xs = outputs[order]
    tcs = tc_np[order]

    los = np.empty(NT, np.int64)
    spans = np.empty(NT, np.int64)
    for t in range(NT):
        seg = tcs[t * 128 : (t + 1) * 128]
        los[t] = seg.min()
        spans[t] = seg.max() - seg.min() + 1
    nb = max(1, int(-(-int(spans.max()) // 16)))
    ncol = 512 * nb
    nw = ncol // 32

    flat = clusters.reshape(C * P, F)
    c2 = (flat.astype(np.float64) ** 2).sum(axis=1).astype(np.float32)

    # -2x in fp8, laid out (p, k, s, r): feature = k*256 + s*128 + p
    a8 = np.clip(-2.0 * xs, -240, 240).astype(np_k)  # [B, F]

    in_maps = []
    for ci in range(NCORES):
        rows = slice(ci * RPC, (ci + 1) * RPC)
        xa_i = np.ascontiguousarray(
            a8[rows].T.reshape(K3, 2, 128, RPC).transpose(2, 0, 1, 3)
            .reshape(128, K3 * 2 * RPC)
        )

        cg_i = np.zeros((128, K3, TPC, 2, ncol), np_k)
        mb_i = np.zeros((1, TPC * 2 * ncol + 256), np_k)
        m2_i = np.zeros((1, TPC * ncol + 128), np_b)
        mk_i = np.zeros((128, TPC * nw), np.float32)
        for tt in range(TPC):
            t = ci * TPC + tt
            lo = int(los[t])
            hi = min(lo + nw, C)
            npro = (hi - lo) * P
            G = flat[lo * P : hi * P]                       # [npro, F]
            g8 = np.clip(G, -240, 240).astype(np_k)
            # (F, npro) -> (k, s, p, npro) -> (p, k, s, npro)
            cg_i[:, :, tt, :, :npro] = (
                g8.T.reshape(K3, 2, 128, npro).transpose(2, 0, 1, 3)
            )
            c2t = np.zeros(ncol, np.float32)
            c2t[:npro] = c2[lo * P : hi * P]
            h8 = np.clip(c2t / 16.0, -240, 240).astype(np_k)
            r8 = np.clip(c2t - 16.0 * h8.astype(np.float32), -240, 240).astype(np_k)
            mb_i[0, tt * 2 * ncol : tt * 2 * ncol + ncol] = h8
            mb_i[0, tt * 2 * ncol + ncol : (tt + 1) * 2 * ncol] = r8
            m2_i[0, tt * ncol : (tt + 1) * ncol] = c2t.astype(np_b)
            w = tcs[t * 128 : (t + 1) * 128] - lo           # [128] window idx
            mk_i[np.arange(128), tt * nw + w] = 1.0
        mb_i[0, TPC * 2 * ncol : TPC * 2 * ncol + 128] = np.float32(16.0).astype(np_k)
        mb_i[0, TPC * 2 * ncol + 128 :] = np.float32(1.0).astype(np_k)
        m2_i[0, TPC * ncol :] = np.float32(1.0).astype(np_b)

        im = {
            "xa": xa_i,
            "cg": np.ascontiguousarray(cg_i.reshape(128, -1)),
            "mb": mb_i,
            "mk": mk_i,
        }
        if V_C2 == "bf16":
            im["m2"] = m2_i
        in_maps.append(im)
    return nb, in_maps


def _finish(results):
    s = 0.0
    for r in results:
        o = r["out"].astype(np.float64)
        s += o[:, 0].sum() + o[:, 1].sum() + o[:, 2].sum() / 4.0
    t = np.float32(s / (B * F))
    ans = np.float32(ALPHA) * t + np.float32(BETA) * (np.float32(1.0) - t)
    return np.asarray(ans, dtype=np.float32)


def kernel(outputs, clusters, target_classes, _run_kwargs=None):
    nb, in_maps = _prep_inputs(outputs, clusters, target_classes)
    nc = _build_program(nb)
    kw = _run_kwargs or {}
    res = run_bass_kernel_spmd(nc, in_maps, list(range(NCORES)), **kw)
    ans = _finish(res.results)
    if _run_kwargs is not None:
        kernel.last_result = res
    return ans


if __name__ == "__main__":
    rng = np.random.default_rng(0)
    o = rng.standard_normal((B, F), dtype=np.float32)
    cl = rng.standard_normal((C, P, F), dtype=np.float32)
    t = rng.integers(0, C, size=(B,)).astype(np.int32)
    print(kernel(o, cl, t))
